# revision 13
# baseline (speedup 1.0000x reference)
"""Trainium2 Bass kernel for nn_AttentionLayer (B=64, F=1024, K=1024).

Reference computation (per batch b):
    scores[k, g] = sum_f input[b, f, k] * weight[f, g] + bias[g]
    alpha        = softmax(scores, axis=g)
    out[b, f, k] = input[b, f, k] * alpha[k, f]

Strategy: data-parallel over batch across 8 NeuronCores (8 batches/core).
Per batch, everything is computed in the transposed [g, k] layout so that no
transposes are ever needed:
    scoresT[g, k] = sum_f W[f, g] * X[f, k]      (lhsT = W chunk, rhs = X chunk)
    E[g, k]  = exp(scoresT + bias[g])            (ScalarE, bias per-partition,
                                                  bf16 out)
    T[g, k]  = sum over the 8 g-chunk tiles      (7 DVE adds, fp32 accum,
                                                  hidden under the matmuls)
    S[., k]  = sum_g T[g, k]                     (ONE ones-matmul -> sum
                                                  replicated across partitions)
    D = 1/S  (reciprocal_approx_fast, DVE)
    P[g, k]  = X * E                             (DVE bf16 2x mode)
    out[f,k] = P[f, k] * D[k]                    (Pool/GPSIMD engine, bf16 out;
                                                  host upcasts to fp32)

v2 vs the fp32r baseline: X and W stream in as bf16 (host-converted), halving
DMA-in bytes and SBUF footprint; matmuls run bf16 (same 1 cyc/row as fp32r);
E is bf16 so the X*E muls run in DVE 2x mode; the final *D muls move to the
otherwise-idle Pool engine. PE (~225us busy: 64 main matmuls + 1 ones-matmul
per half-batch slab) becomes the single bottleneck; DVE ~9us/slab, Act
~6us/slab, Pool ~4-9us/slab, DMA ~9us/slab all hide under PE's 13.9us/slab.
The prev slab's epilogue (ones-matmul after main group 1, recip, Pool muls +
DMA-out after group 3) is interleaved into the current slab's matmul stream.
"""

import sys
from contextlib import ExitStack

import numpy as np

for _p in ("/opt/trn_rl_repo", "/root/.axon_site/_ro/trn_rl_repo"):
    if _p not in sys.path:
        sys.path.append(_p)

import concourse.bacc as bacc
import concourse.bass as bass
import concourse.mybir as mybir
import concourse.tile as tile
from concourse.bass_utils import run_bass_kernel_spmd

N_CORES = 8
B, F, K = 64, 1024, 1024
BPC = B // N_CORES            # batches per core
P = 128                       # SBUF partitions
NF = F // P                   # f (contraction) chunks
NG = F // P                   # g (feature/output-partition) chunks
KC = 512                      # moving free-dim chunk (max moving = 512)
NK = K // KC

FP32 = mybir.dt.float32
F32R = mybir.dt.float32r
BF16 = mybir.dt.bfloat16

EXP = mybir.ActivationFunctionType.Exp


def _build(bpc: int = BPC, reps: int = 1):
    nc = bacc.Bacc("TRN2", target_bir_lowering=False, debug=False)

    x_d = nc.dram_tensor("x", [bpc, F, K], BF16, kind="ExternalInput").ap()
    w_d = nc.dram_tensor("w", [F, F], BF16, kind="ExternalInput").ap()
    b_d = nc.dram_tensor("b", [1, F], FP32, kind="ExternalInput").ap()
    ones_d = nc.dram_tensor("ones", [P, P], F32R, kind="ExternalInput").ap()
    o_d = nc.dram_tensor("out", [bpc, F, K], BF16, kind="ExternalOutput").ap()

    with tile.TileContext(nc) as tc, ExitStack() as ctx:
        w_pool = ctx.enter_context(tc.tile_pool(name="w", bufs=1))
        c_pool = ctx.enter_context(tc.tile_pool(name="const", bufs=1))
        x_pool = ctx.enter_context(tc.tile_pool(name="x", bufs=16))
        e_pool = ctx.enter_context(tc.tile_pool(name="e", bufs=12))
        p_pool = ctx.enter_context(tc.tile_pool(name="pp", bufs=18))
        t_pool = ctx.enter_context(tc.tile_pool(name="t", bufs=3))
        d_pool = ctx.enter_context(tc.tile_pool(name="d", bufs=3))
        o_pool = ctx.enter_context(tc.tile_pool(name="o", bufs=10))
        sc_psum = ctx.enter_context(tc.tile_pool(name="sc", bufs=6, space="PSUM"))
        s_psum = ctx.enter_context(tc.tile_pool(name="s", bufs=2, space="PSUM"))

        # ---- constants; DMAs emitted inside the first prefetch, AFTER
        # batch-0's critical chunks (bias/ones aren't read until ~17us)
        bias_sb = c_pool.tile([P, NG], FP32)
        ones_sb = c_pool.tile([P, P], F32R)

        def load_consts():
            nc.sync.dma_start(
                out=bias_sb[:], in_=b_d.rearrange("o (c p) -> (o p) c", p=P)
            )
            nc.sync.dma_start(out=ones_sb[:], in_=ones_d)

        # w_sb[p, fc*F + g] = W[fc*128 + p, g]
        w_sb = w_pool.tile([P, NF * F], BF16)

        def w_tile(fc, gc):
            off = fc * F + gc * P
            return w_sb[:, off : off + P]

        def prefetch_x(b, with_w=False):
            x_tiles = []
            for fc in range(NF):
                x_t = x_pool.tile([P, K], BF16, tag="x")
                if with_w:
                    # startup: interleave the g-low half of W with batch-0's
                    # kc=0 X halves so slab 0's first matmul groups start as
                    # early as possible; the rest streams in behind them
                    nc.sync.dma_start(
                        out=w_sb[:, fc * F : fc * F + F // 2],
                        in_=w_d[fc * P : (fc + 1) * P, 0 : F // 2],
                    )
                    nc.sync.dma_start(
                        out=x_t[:, 0:KC], in_=x_d[b, fc * P : (fc + 1) * P, 0:KC]
                    )
                else:
                    nc.sync.dma_start(
                        out=x_t[:], in_=x_d[b, fc * P : (fc + 1) * P, :]
                    )
                x_tiles.append(x_t)
            if with_w:
                load_consts()
                for fc in range(NF):
                    nc.sync.dma_start(
                        out=w_sb[:, fc * F + F // 2 : (fc + 1) * F],
                        in_=w_d[fc * P : (fc + 1) * P, F // 2 : F],
                    )
                for fc in range(NF):
                    nc.sync.dma_start(
                        out=x_tiles[fc][:, KC:K],
                        in_=x_d[b, fc * P : (fc + 1) * P, KC:K],
                    )
            return x_tiles

        def emit_sum(prev):
            """Prev slab: partition-sum ones-matmul + reciprocal -> D."""
            _, _, _, t_t = prev
            s_t = s_psum.tile([P, KC], FP32, tag="s")
            nc.tensor.matmul(
                s_t[:], lhsT=ones_sb[:], rhs=t_t[:], start=True, stop=True
            )
            d_t = d_pool.tile([P, KC], FP32, tag="d")
            nc.vector.reciprocal_approx_fast(d_t[:], s_t[:])
            return d_t

        def emit_out(prev, d_t):
            """Prev slab: final scale on Pool + DMA out."""
            b, kc, p_tiles, _ = prev
            for fc in range(NF):
                o_t = o_pool.tile([P, KC], BF16, tag="o")
                nc.gpsimd.tensor_mul(o_t[:], p_tiles[fc][:], d_t[:])
                nc.sync.dma_start(
                    out=o_d[b, fc * P : (fc + 1) * P, kc * KC : (kc + 1) * KC],
                    in_=o_t[:],
                )

        def slab_main(b, kc, x_tiles, prev):
            """Main matmuls + exp + E-sum + X*E for one (batch, k-half) slab.

            The prev slab's epilogue is injected into the matmul stream: the
            ones-matmul goes after main group 1 (so PE never waits on prev's
            last DVE add), the Pool muls + output DMA after group 3.
            """
            ks = slice(kc * KC, (kc + 1) * KC)
            e_tiles = []
            p_tiles = []
            t_t = None
            d_prev = None
            for gc in range(NG):
                sc = sc_psum.tile([P, KC], FP32, tag="sc")
                for fc in range(NF):
                    nc.tensor.matmul(
                        sc[:],
                        lhsT=w_tile(fc, gc),
                        rhs=x_tiles[fc][:, ks],
                        start=(fc == 0),
                        stop=(fc == NF - 1),
                    )
                if gc == 2 and prev is not None:
                    d_prev = emit_sum(prev)
                e_t = e_pool.tile([P, KC], BF16, tag="e")
                nc.scalar.activation(
                    e_t[:], sc[:], EXP, bias=bias_sb[:, gc : gc + 1], scale=1.0
                )
                e_tiles.append(e_t)
                # accumulate exp tiles on DVE (runs in the shadow of the
                # matmuls); bf16 inputs, fp32-width accumulator typed f32r
                # so the ones-matmul may consume it directly
                if gc == 1:
                    t_t = t_pool.tile([P, KC], F32R, tag="t")
                    nc.vector.tensor_add(t_t[:], e_tiles[0][:], e_t[:])
                elif gc > 1:
                    nc.vector.tensor_add(
                        t_t[:], t_t[:].bitcast(FP32), e_t[:]
                    )
                # P = X * E needs no denominator -> bf16 2x mode, in the
                # shadow of the matmuls
                p_t = p_pool.tile([P, KC], BF16, tag="pp")
                nc.vector.tensor_mul(p_t[:], x_tiles[gc][:, ks], e_t[:])
                p_tiles.append(p_t)
                if gc == 4 and prev is not None:
                    emit_out(prev, d_prev)
            return (b, kc, p_tiles, t_t)

        # software pipeline over half-batch slabs: each slab's epilogue
        # (partition sum, recip, final scale, store) rides inside the next
        # slab's matmul stream, so the kernel tail after the very last main
        # matmul is only one slab's epilogue.
        prev = None
        first = True
        for _ in range(reps):
            for b in range(bpc):
                x_tiles = prefetch_x(b, with_w=first)
                first = False
                for kc in range(NK):
                    prev = slab_main(b, kc, x_tiles, prev)
        d_last = emit_sum(prev)
        emit_out(prev, d_last)

    nc.compile()
    return nc


_NC = None


def _get_nc():
    global _NC
    if _NC is None:
        _NC = _build()
    return _NC


def kernel(**inputs) -> np.ndarray:
    import ml_dtypes

    x = np.asarray(inputs["input"], dtype=np.float32)
    w = np.asarray(inputs["weight"], dtype=np.float32)
    b = np.ascontiguousarray(np.asarray(inputs["bias"], dtype=np.float32))

    xb = np.ascontiguousarray(x.astype(ml_dtypes.bfloat16))
    wb = np.ascontiguousarray(w.astype(ml_dtypes.bfloat16))

    nc = _get_nc()
    ones = np.ones((P, P), dtype=np.float32)
    in_maps = [
        {"x": xb[c * BPC : (c + 1) * BPC], "w": wb, "b": b, "ones": ones}
        for c in range(N_CORES)
    ]
    res = run_bass_kernel_spmd(nc, in_maps, list(range(N_CORES)))
    return np.concatenate(
        [res.results[c]["out"].astype(np.float32) for c in range(N_CORES)], axis=0
    )


# revision 15
# speedup vs baseline: 1.2011x; 1.2011x over previous
"""Trainium2 Bass kernel for nn_AttentionLayer (B=64, F=1024, K=1024).

Reference computation (per batch b):
    scores[k, g] = sum_f input[b, f, k] * weight[f, g] + bias[g]
    alpha        = softmax(scores, axis=g)
    out[b, f, k] = input[b, f, k] * alpha[k, f]

Strategy: data-parallel over batch across 8 NeuronCores (8 batches/core).
Per batch, everything is computed in the transposed [g, k] layout so that no
transposes are ever needed:
    scoresT[g, k] = sum_f W[f, g] * X[f, k]      (lhsT = W chunk, rhs = X chunk)
    E[g, k]  = exp(scoresT + bias[g])            (ScalarE, bias per-partition,
                                                  bf16 out)
    T[g, k]  = sum over the 8 g-chunk tiles      (7 DVE adds, fp32 accum,
                                                  hidden under the matmuls)
    S[., k]  = sum_g T[g, k]                     (ONE ones-matmul -> sum
                                                  replicated across partitions)
    D = 1/S  (reciprocal_approx_fast, DVE)
    P[g, k]  = X * E                             (DVE bf16 2x mode)
    out[f,k] = P[f, k] * D[k]                    (Pool/GPSIMD engine, bf16 out;
                                                  host upcasts to fp32)

v2 vs the fp32r baseline: X and W stream in as bf16 (host-converted), halving
DMA-in bytes and SBUF footprint; matmuls run bf16 (same 1 cyc/row as fp32r);
E is bf16 so the X*E muls run in DVE 2x mode; the final *D muls move to the
otherwise-idle Pool engine. PE (~225us busy: 64 main matmuls + 1 ones-matmul
per half-batch slab) becomes the single bottleneck; DVE ~9us/slab, Act
~6us/slab, Pool ~4-9us/slab, DMA ~9us/slab all hide under PE's 13.9us/slab.
The prev slab's epilogue (ones-matmul after main group 1, recip, Pool muls +
DMA-out after group 3) is interleaved into the current slab's matmul stream.
"""

import sys
from contextlib import ExitStack

import numpy as np

for _p in ("/opt/trn_rl_repo", "/root/.axon_site/_ro/trn_rl_repo"):
    if _p not in sys.path:
        sys.path.append(_p)

import concourse.bacc as bacc
import concourse.bass as bass
import concourse.mybir as mybir
import concourse.tile as tile
from concourse.bass_utils import run_bass_kernel_spmd

N_CORES = 8
B, F, K = 64, 1024, 1024
BPC = B // N_CORES            # batches per core
P = 128                       # SBUF partitions
NF = F // P                   # f (contraction) chunks
NG = F // P                   # g (feature/output-partition) chunks
KC = 512                      # moving free-dim chunk (max moving = 512)
NK = K // KC

FP32 = mybir.dt.float32
F32R = mybir.dt.float32r
BF16 = mybir.dt.bfloat16

EXP = mybir.ActivationFunctionType.Exp


def _build(bpc: int = BPC, reps: int = 1):
    nc = bacc.Bacc("TRN2", target_bir_lowering=False, debug=False)

    x_d = nc.dram_tensor("x", [bpc, F, K], BF16, kind="ExternalInput").ap()
    w_d = nc.dram_tensor("w", [F, F], BF16, kind="ExternalInput").ap()
    b_d = nc.dram_tensor("b", [1, F], FP32, kind="ExternalInput").ap()
    ones_d = nc.dram_tensor("ones", [P, P], F32R, kind="ExternalInput").ap()
    o_d = nc.dram_tensor("out", [bpc, F, K], BF16, kind="ExternalOutput").ap()

    with tile.TileContext(nc) as tc, ExitStack() as ctx:
        w_pool = ctx.enter_context(tc.tile_pool(name="w", bufs=1))
        c_pool = ctx.enter_context(tc.tile_pool(name="const", bufs=1))
        x_pool = ctx.enter_context(tc.tile_pool(name="x", bufs=16))
        e_pool = ctx.enter_context(tc.tile_pool(name="e", bufs=12))
        p_pool = ctx.enter_context(tc.tile_pool(name="pp", bufs=18))
        t_pool = ctx.enter_context(tc.tile_pool(name="t", bufs=3))
        d_pool = ctx.enter_context(tc.tile_pool(name="d", bufs=3))
        o_pool = ctx.enter_context(tc.tile_pool(name="o", bufs=10))
        sc_psum = ctx.enter_context(tc.tile_pool(name="sc", bufs=6, space="PSUM"))
        s_psum = ctx.enter_context(tc.tile_pool(name="s", bufs=2, space="PSUM"))

        # ---- constants; DMAs emitted inside the first prefetch, AFTER
        # batch-0's critical chunks (bias/ones aren't read until ~17us)
        bias_sb = c_pool.tile([P, NG], FP32)
        ones_sb = c_pool.tile([P, P], F32R)

        def load_consts():
            nc.sync.dma_start(
                out=bias_sb[:], in_=b_d.rearrange("o (c p) -> (o p) c", p=P)
            )
            nc.sync.dma_start(out=ones_sb[:], in_=ones_d)

        # w_sb[p, fc*F + g] = W[fc*128 + p, g]
        w_sb = w_pool.tile([P, NF * F], BF16)

        def w_tile(fc, gc):
            off = fc * F + gc * P
            return w_sb[:, off : off + P]

        def prefetch_x(b, with_w=False):
            x_tiles = []
            for fc in range(NF):
                x_t = x_pool.tile([P, K], BF16, tag="x")
                if with_w:
                    # startup: interleave the g-low half of W with batch-0's
                    # kc=0 X halves so slab 0's first matmul groups start as
                    # early as possible; the rest streams in behind them
                    nc.sync.dma_start(
                        out=w_sb[:, fc * F : fc * F + F // 2],
                        in_=w_d[fc * P : (fc + 1) * P, 0 : F // 2],
                    )
                    nc.sync.dma_start(
                        out=x_t[:, 0:KC], in_=x_d[b, fc * P : (fc + 1) * P, 0:KC]
                    )
                else:
                    nc.sync.dma_start(
                        out=x_t[:], in_=x_d[b, fc * P : (fc + 1) * P, :]
                    )
                x_tiles.append(x_t)
            if with_w:
                load_consts()
                for fc in range(NF):
                    nc.sync.dma_start(
                        out=w_sb[:, fc * F + F // 2 : (fc + 1) * F],
                        in_=w_d[fc * P : (fc + 1) * P, F // 2 : F],
                    )
                for fc in range(NF):
                    nc.sync.dma_start(
                        out=x_tiles[fc][:, KC:K],
                        in_=x_d[b, fc * P : (fc + 1) * P, KC:K],
                    )
            return x_tiles

        def emit_sum(prev):
            """Prev slab: partition-sum ones-matmul + reciprocal -> D."""
            _, _, _, t_t = prev
            s_t = s_psum.tile([P, KC], FP32, tag="s")
            nc.tensor.matmul(
                s_t[:], lhsT=ones_sb[:], rhs=t_t[:], start=True, stop=True
            )
            d_t = d_pool.tile([P, KC], FP32, tag="d")
            nc.vector.reciprocal_approx_fast(d_t[:], s_t[:])
            return d_t

        def emit_out(prev, d_t, tail=False):
            """Prev slab: final scale on Pool + DMA out.

            In the drain tail there are no matmuls left to hide behind, so
            split the muls across Pool and the now-idle DVE.
            """
            b, kc, p_tiles, _ = prev
            for fc in range(NF):
                o_t = o_pool.tile([P, KC], BF16, tag="o")
                eng = nc.vector if (tail and fc % 2) else nc.gpsimd
                eng.tensor_mul(o_t[:], p_tiles[fc][:], d_t[:])
                nc.sync.dma_start(
                    out=o_d[b, fc * P : (fc + 1) * P, kc * KC : (kc + 1) * KC],
                    in_=o_t[:],
                )

        def slab_main(b, kc, x_tiles, prev):
            """Main matmuls + exp + E-sum + X*E for one (batch, k-half) slab.

            The prev slab's epilogue is injected into the matmul stream: the
            ones-matmul goes after main group 1 (so PE never waits on prev's
            last DVE add), the Pool muls + output DMA after group 3.
            """
            ks = slice(kc * KC, (kc + 1) * KC)
            e_tiles = []
            p_tiles = []
            t_t = None
            d_prev = None
            for gc in range(NG):
                sc = sc_psum.tile([P, KC], FP32, tag="sc")
                for fc in range(NF):
                    nc.tensor.matmul(
                        sc[:],
                        lhsT=w_tile(fc, gc),
                        rhs=x_tiles[fc][:, ks],
                        start=(fc == 0),
                        stop=(fc == NF - 1),
                    )
                if gc == 2 and prev is not None:
                    d_prev = emit_sum(prev)
                e_t = e_pool.tile([P, KC], BF16, tag="e")
                nc.scalar.activation(
                    e_t[:], sc[:], EXP, bias=bias_sb[:, gc : gc + 1], scale=1.0
                )
                e_tiles.append(e_t)
                # accumulate exp tiles on DVE (runs in the shadow of the
                # matmuls); bf16 inputs, fp32-width accumulator typed f32r
                # so the ones-matmul may consume it directly
                if gc == 1:
                    t_t = t_pool.tile([P, KC], F32R, tag="t")
                    nc.vector.tensor_add(t_t[:], e_tiles[0][:], e_t[:])
                elif gc > 1:
                    nc.vector.tensor_add(
                        t_t[:], t_t[:].bitcast(FP32), e_t[:]
                    )
                # P = X * E needs no denominator -> bf16 2x mode, in the
                # shadow of the matmuls
                p_t = p_pool.tile([P, KC], BF16, tag="pp")
                nc.vector.tensor_mul(p_t[:], x_tiles[gc][:, ks], e_t[:])
                p_tiles.append(p_t)
                if gc == 4 and prev is not None:
                    emit_out(prev, d_prev)
            return (b, kc, p_tiles, t_t)

        # software pipeline over half-batch slabs: each slab's epilogue
        # (partition sum, recip, final scale, store) rides inside the next
        # slab's matmul stream, so the kernel tail after the very last main
        # matmul is only one slab's epilogue. X is prefetched one batch
        # ahead (issued after the first slab of the preceding batch) so
        # batch boundaries never wait on input DMA even when HBM is slow.
        seq = [b for _ in range(reps) for b in range(bpc)]
        prev = None
        x_cur = prefetch_x(seq[0], with_w=True)
        x_nxt = None
        for i, b in enumerate(seq):
            for kc in range(NK):
                prev = slab_main(b, kc, x_cur, prev)
                if kc == 0 and i + 1 < len(seq):
                    x_nxt = prefetch_x(seq[i + 1])
            x_cur = x_nxt
        d_last = emit_sum(prev)
        emit_out(prev, d_last, tail=True)

    nc.compile()
    return nc


_NC = None


def _get_nc():
    global _NC
    if _NC is None:
        _NC = _build()
    return _NC


def kernel(**inputs) -> np.ndarray:
    import ml_dtypes

    x = np.asarray(inputs["input"], dtype=np.float32)
    w = np.asarray(inputs["weight"], dtype=np.float32)
    b = np.ascontiguousarray(np.asarray(inputs["bias"], dtype=np.float32))

    xb = np.ascontiguousarray(x.astype(ml_dtypes.bfloat16))
    wb = np.ascontiguousarray(w.astype(ml_dtypes.bfloat16))

    nc = _get_nc()
    ones = np.ones((P, P), dtype=np.float32)
    in_maps = [
        {"x": xb[c * BPC : (c + 1) * BPC], "w": wb, "b": b, "ones": ones}
        for c in range(N_CORES)
    ]
    res = run_bass_kernel_spmd(nc, in_maps, list(range(N_CORES)))
    return np.concatenate(
        [res.results[c]["out"].astype(np.float32) for c in range(N_CORES)], axis=0
    )


# revision 21
# speedup vs baseline: 1.2334x; 1.0269x over previous
"""Trainium2 Bass kernel for nn_AttentionLayer (B=64, F=1024, K=1024).

Reference computation (per batch b):
    scores[k, g] = sum_f input[b, f, k] * weight[f, g] + bias[g]
    alpha        = softmax(scores, axis=g)
    out[b, f, k] = input[b, f, k] * alpha[k, f]

Strategy: data-parallel over batch across 8 NeuronCores (8 batches/core).
Per batch, everything is computed in the transposed [g, k] layout so that no
transposes are ever needed:
    scoresT[g, k] = sum_f W[f, g] * X[f, k]      (lhsT = W chunk, rhs = X chunk)
    E[g, k]  = exp(scoresT + bias[g])            (ScalarE, bias per-partition,
                                                  bf16 out)
    T[g, k]  = sum over the 8 g-chunk tiles      (7 DVE adds, fp32 accum,
                                                  hidden under the matmuls)
    S[., k]  = sum_g T[g, k]                     (ONE ones-matmul -> sum
                                                  replicated across partitions)
    D = 1/S  (reciprocal_approx_fast, DVE)
    P[g, k]  = X * E                             (DVE bf16 2x mode)
    out[f,k] = P[f, k] * D[k]                    (Pool/GPSIMD engine, bf16 out;
                                                  host upcasts to fp32)

v2 vs the fp32r baseline: X and W stream in as bf16 (host-converted), halving
DMA-in bytes and SBUF footprint; matmuls run bf16 (same 1 cyc/row as fp32r);
E is bf16 so the X*E muls run in DVE 2x mode; the final *D muls move to the
otherwise-idle Pool engine. PE (~225us busy: 64 main matmuls + 1 ones-matmul
per half-batch slab) becomes the single bottleneck; DVE ~9us/slab, Act
~6us/slab, Pool ~4-9us/slab, DMA ~9us/slab all hide under PE's 13.9us/slab.
The prev slab's epilogue (ones-matmul after main group 1, recip, Pool muls +
DMA-out after group 3) is interleaved into the current slab's matmul stream.
"""

import sys
from contextlib import ExitStack

import numpy as np

for _p in ("/opt/trn_rl_repo", "/root/.axon_site/_ro/trn_rl_repo"):
    if _p not in sys.path:
        sys.path.append(_p)

import concourse.bacc as bacc
import concourse.bass as bass
import concourse.mybir as mybir
import concourse.tile as tile
from concourse.bass_utils import run_bass_kernel_spmd

N_CORES = 8
B, F, K = 64, 1024, 1024
BPC = B // N_CORES            # batches per core
P = 128                       # SBUF partitions
NF = F // P                   # f (contraction) chunks
NG = F // P                   # g (feature/output-partition) chunks
KC = 512                      # moving free-dim chunk (max moving = 512)
NK = K // KC

FP32 = mybir.dt.float32
F32R = mybir.dt.float32r
BF16 = mybir.dt.bfloat16

EXP = mybir.ActivationFunctionType.Exp


def _build(bpc: int = BPC, reps: int = 1):
    nc = bacc.Bacc("TRN2", target_bir_lowering=False, debug=False)

    x_d = nc.dram_tensor("x", [bpc, F, K], BF16, kind="ExternalInput").ap()
    w_d = nc.dram_tensor("w", [F, F], BF16, kind="ExternalInput").ap()
    b_d = nc.dram_tensor("b", [1, F], FP32, kind="ExternalInput").ap()
    ones_d = nc.dram_tensor("ones", [P, P], F32R, kind="ExternalInput").ap()
    o_d = nc.dram_tensor("out", [bpc, F, K], BF16, kind="ExternalOutput").ap()

    with tile.TileContext(nc) as tc, ExitStack() as ctx:
        w_pool = ctx.enter_context(tc.tile_pool(name="w", bufs=1))
        c_pool = ctx.enter_context(tc.tile_pool(name="const", bufs=1))
        x_pool = ctx.enter_context(tc.tile_pool(name="x", bufs=16))
        e_pool = ctx.enter_context(tc.tile_pool(name="e", bufs=12))
        p_pool = ctx.enter_context(tc.tile_pool(name="pp", bufs=18))
        t_pool = ctx.enter_context(tc.tile_pool(name="t", bufs=3))
        d_pool = ctx.enter_context(tc.tile_pool(name="d", bufs=3))
        o_pool = ctx.enter_context(tc.tile_pool(name="o", bufs=10))
        sc_psum = ctx.enter_context(tc.tile_pool(name="sc", bufs=6, space="PSUM"))
        s_psum = ctx.enter_context(tc.tile_pool(name="s", bufs=2, space="PSUM"))

        # ---- constants; DMAs emitted inside the first prefetch, AFTER
        # batch-0's critical chunks (bias/ones aren't read until ~17us)
        bias_sb = c_pool.tile([P, NG], FP32)
        ones_sb = c_pool.tile([P, P], F32R)

        def load_bias():
            nc.sync.dma_start(
                out=bias_sb[:], in_=b_d.rearrange("o (c p) -> (o p) c", p=P)
            )

        def load_ones():
            nc.sync.dma_start(out=ones_sb[:], in_=ones_d)

        # w_sb[p, fc*F + g] = W[fc*128 + p, g]
        w_sb = w_pool.tile([P, NF * F], BF16)

        def w_tile(fc, gc):
            off = fc * F + gc * P
            return w_sb[:, off : off + P]

        def prefetch_x(b, with_w=False):
            x_tiles = []
            for fc in range(NF):
                x_t = x_pool.tile([P, K], BF16, tag="x")
                if with_w:
                    # startup: interleave the g-low half of W with batch-0's
                    # kc=0 X halves so slab 0's first matmul groups start as
                    # early as possible; the rest streams in behind them
                    nc.sync.dma_start(
                        out=w_sb[:, fc * F : fc * F + F // 2],
                        in_=w_d[fc * P : (fc + 1) * P, 0 : F // 2],
                    )
                    nc.sync.dma_start(
                        out=x_t[:, 0:KC], in_=x_d[b, fc * P : (fc + 1) * P, 0:KC]
                    )
                else:
                    nc.sync.dma_start(
                        out=x_t[:], in_=x_d[b, fc * P : (fc + 1) * P, :]
                    )
                x_tiles.append(x_t)
            if with_w:
                # bias is tiny and gates the first exp -> goes first; W's
                # g-high half must beat the first slab's gc>=4 matmul groups;
                # ones isn't read until the second slab
                load_bias()
                for fc in range(NF):
                    nc.sync.dma_start(
                        out=w_sb[:, fc * F + F // 2 : (fc + 1) * F],
                        in_=w_d[fc * P : (fc + 1) * P, F // 2 : F],
                    )
                load_ones()
                for fc in range(NF):
                    nc.sync.dma_start(
                        out=x_tiles[fc][:, KC:K],
                        in_=x_d[b, fc * P : (fc + 1) * P, KC:K],
                    )
            return x_tiles

        def emit_sum(prev):
            """Prev slab: partition-sum ones-matmul + reciprocal -> D."""
            _, _, _, t_t = prev
            s_t = s_psum.tile([P, KC], FP32, tag="s")
            nc.tensor.matmul(
                s_t[:], lhsT=ones_sb[:], rhs=t_t[:], start=True, stop=True
            )
            d_t = d_pool.tile([P, KC], FP32, tag="d")
            nc.vector.reciprocal_approx_fast(d_t[:], s_t[:])
            return d_t

        def emit_out(prev, d_t, tail=False):
            """Prev slab: final scale on Pool + DMA out.

            In the drain tail there are no matmuls left to hide behind, so
            split the muls across Pool and the now-idle DVE.
            """
            b, kc, p_tiles, _ = prev
            for fc in range(NF):
                o_t = o_pool.tile([P, KC], BF16, tag="o")
                eng = nc.vector if (tail and fc % 2) else nc.gpsimd
                eng.tensor_mul(o_t[:], p_tiles[fc][:], d_t[:])
                nc.sync.dma_start(
                    out=o_d[b, fc * P : (fc + 1) * P, kc * KC : (kc + 1) * KC],
                    in_=o_t[:],
                )

        def slab_main(b, kc, x_tiles, prev):
            """Main matmuls + exp + E-sum + X*E for one (batch, k-half) slab.

            The prev slab's epilogue is injected into the matmul stream: the
            ones-matmul goes after main group 1 (so PE never waits on prev's
            last DVE add), the Pool muls + output DMA after group 3.
            """
            ks = slice(kc * KC, (kc + 1) * KC)
            e_tiles = []
            p_tiles = []
            t_t = None
            d_prev = None
            for gc in range(NG):
                sc = sc_psum.tile([P, KC], FP32, tag="sc")
                for fc in range(NF):
                    nc.tensor.matmul(
                        sc[:],
                        lhsT=w_tile(fc, gc),
                        rhs=x_tiles[fc][:, ks],
                        start=(fc == 0),
                        stop=(fc == NF - 1),
                    )
                if gc == 2 and prev is not None:
                    d_prev = emit_sum(prev)
                e_t = e_pool.tile([P, KC], BF16, tag="e")
                nc.scalar.activation(
                    e_t[:], sc[:], EXP, bias=bias_sb[:, gc : gc + 1], scale=1.0
                )
                e_tiles.append(e_t)
                # accumulate exp tiles on DVE (runs in the shadow of the
                # matmuls); bf16 inputs, fp32-width accumulator typed f32r
                # so the ones-matmul may consume it directly
                if gc == 1:
                    t_t = t_pool.tile([P, KC], F32R, tag="t")
                    nc.vector.tensor_add(t_t[:], e_tiles[0][:], e_t[:])
                elif gc > 1:
                    nc.vector.tensor_add(
                        t_t[:], t_t[:].bitcast(FP32), e_t[:]
                    )
                # P = X * E needs no denominator -> bf16 2x mode, in the
                # shadow of the matmuls
                p_t = p_pool.tile([P, KC], BF16, tag="pp")
                nc.vector.tensor_mul(p_t[:], x_tiles[gc][:, ks], e_t[:])
                p_tiles.append(p_t)
                if gc == 4 and prev is not None:
                    emit_out(prev, d_prev)
            return (b, kc, p_tiles, t_t)

        # software pipeline over half-batch slabs: each slab's epilogue
        # (partition sum, recip, final scale, store) rides inside the next
        # slab's matmul stream, so the kernel tail after the very last main
        # matmul is only one slab's epilogue. X is prefetched one batch
        # ahead (issued after the first slab of the preceding batch) so
        # batch boundaries never wait on input DMA even when HBM is slow.
        seq = [b for _ in range(reps) for b in range(bpc)]
        prev = None
        x_cur = prefetch_x(seq[0], with_w=True)
        x_nxt = None
        for i, b in enumerate(seq):
            for kc in range(NK):
                prev = slab_main(b, kc, x_cur, prev)
                if kc == 0 and i + 1 < len(seq):
                    x_nxt = prefetch_x(seq[i + 1])
            x_cur = x_nxt
        d_last = emit_sum(prev)
        emit_out(prev, d_last, tail=True)

    nc.compile()
    return nc


_NC = None


def _get_nc():
    global _NC
    if _NC is None:
        _NC = _build()
    return _NC


def kernel(**inputs) -> np.ndarray:
    import ml_dtypes

    x = np.asarray(inputs["input"], dtype=np.float32)
    w = np.asarray(inputs["weight"], dtype=np.float32)
    b = np.ascontiguousarray(np.asarray(inputs["bias"], dtype=np.float32))

    xb = np.ascontiguousarray(x.astype(ml_dtypes.bfloat16))
    wb = np.ascontiguousarray(w.astype(ml_dtypes.bfloat16))

    nc = _get_nc()
    ones = np.ones((P, P), dtype=np.float32)
    in_maps = [
        {"x": xb[c * BPC : (c + 1) * BPC], "w": wb, "b": b, "ones": ones}
        for c in range(N_CORES)
    ]
    res = run_bass_kernel_spmd(nc, in_maps, list(range(N_CORES)))
    return np.concatenate(
        [res.results[c]["out"].astype(np.float32) for c in range(N_CORES)], axis=0
    )
